# revision 10
# baseline (speedup 1.0000x reference)
"""GCN(2-layer) + DQN-head kernel for 8 TRN2 NeuronCores.

v2: fc1 weights streamed as fp8e4m3 scaled by 2^12 (halves the dominant HBM
stream; the inverse scale folds into the tail activation). W1/W2 scaled by 8
so G1/G2 fp8 values sit in e4m3's normal range (host sim: rel err 1.3e-3 vs
fp32 reference). fc1 matmul is mixed bf16 (e) x fp8 (wf).

Per core c of 8 (owns dst nodes [c*1024, (c+1)*1024)):
  host:   fold deg^-1/2 into x rows; dense (Adj+I) count block
          A[src=8192, dst=1024] fp8; fc1_w slice * 4096 -> fp8 tiles.
  device: G1[src,feat] = xs_bf @ (8*W1)           (fp8 result, x8 scale)
          out1[feat,dst] = sum_src G1[src,:]*A[src,dst]   (A resident)
          h1s = dinvb * relu(out1*dinvb/8 + b1)
          G2 = h1s^T @ (8*W2) -> fp8 -> AllGather -> g_res
          agg2 per 256-dst group; e_bf = bf16(psum*dinvb/8 + b2)
          fc1: [K=128,M=4]x[128,512] e_bf x wf_fp8 matmuls, psum x4096
          diag extract -> AllReduce(128) -> relu(x/4096+b)/fc2/fc3 -> q
"""
import sys
import numpy as np

sys.path.insert(0, "/opt/trn_rl_repo")

import ml_dtypes
import concourse.bass as bass
import concourse.tile as tile
import concourse.mybir as mybir
from concourse import bacc
from concourse.bass_utils import run_bass_kernel_spmd
from concourse.masks import make_identity

F32 = mybir.dt.float32
BF16 = mybir.dt.bfloat16
FP8 = mybir.dt.float8e4

N_CORES = 8
N = 8192
D = 128
NL = N // N_CORES          # 1024 local dst nodes
SC = N // 128              # 64 src chunks
WF_TILE = 16384            # wf tile free dim (128 nodes/tile, fp8 -> 2MB)
WF_TILES = NL * D // WF_TILE   # 8 tiles per core
GROUPS = 4                 # dst groups for agg2/fc1 interleave
GDST = NL // GROUPS        # 256 dst per group

WF_SCALE = 4096.0          # fc1_w stored as fp8(w * 4096)
G_SCALE = 8.0              # W1/W2 stored as bf16(8*W); G2 fp8 holds 8x values
XT_SCALE = 8.0             # x*dinv stored as fp8(8*xs); G1 fp8 holds 64x

_NC_CACHE = {}


def build_nc(reps=1, loops=None):
    key = (reps, loops)
    if key in _NC_CACHE:
        return _NC_CACHE[key]
    nc = bacc.Bacc("TRN2", target_bir_lowering=False, debug=False,
                   num_devices=N_CORES)

    # ---------------- DRAM I/O ----------------
    d_xt = nc.dram_tensor("xt", [2, 128, 4096], FP8, kind="ExternalInput")
    d_a = nc.dram_tensor("ablk", [8, 128, 8192], FP8, kind="ExternalInput")
    d_wf = nc.dram_tensor("wf", [WF_TILES, 128, WF_TILE], FP8,
                          kind="ExternalInput")
    d_dinvb = nc.dram_tensor("dinvb", [128, 3 * NL], F32, kind="ExternalInput")
    d_consts = nc.dram_tensor("consts", [128, 144], F32, kind="ExternalInput")
    d_cbf = nc.dram_tensor("cbf", [128, 256], BF16, kind="ExternalInput")
    d_q = nc.dram_tensor("out_q", [1, 1], F32, kind="ExternalOutput")

    # collective bounce buffers
    ag_in = nc.dram_tensor("ag_in", [NL, D], FP8)
    ag_out = nc.dram_tensor("ag_out", [N, D], FP8, addr_space="Shared")
    ar_in = nc.dram_tensor("ar_in", [1, 128], F32)
    ar_out = nc.dram_tensor("ar_out", [1, 128], F32, addr_space="Shared")

    # consts (f32) columns
    C_FC2W = 0      # [128, 128]
    C_FC3W = 128    # [128, 1]
    C_B1 = 129
    C_B2 = 130
    C_FC1B = 131
    C_FC2B = 132
    C_FC3B = 133    # replicated; use [0:1]
    # cbf (bf16) columns
    CB_W1 = 0       # [128, 128] (8*W1)
    CB_W2 = 128     # [128, 128] (8*W2)

    with tile.TileContext(nc) as tc:
        from contextlib import ExitStack, nullcontext
        with ExitStack() as ctx:
            P = bass.MemorySpace.PSUM
            res = ctx.enter_context(tc.tile_pool(name="res", bufs=1))
            stream = ctx.enter_context(tc.tile_pool(name="stream", bufs=3))
            psum = ctx.enter_context(tc.tile_pool(name="psum", bufs=1, space=P))
            psum2 = ctx.enter_context(tc.tile_pool(name="psum2", bufs=2, space=P))

            # ------- resident tensors -------
            a_res = res.tile([128, SC * NL], FP8, tag="a_res")    # 64KB/part
            g_res = res.tile([128, N], FP8, tag="g_res")          # 8KB/part
            e_bf = res.tile([128, NL], BF16, tag="e_bf")
            h1_sb = res.tile([128, NL], F32, tag="h1")
            dinvb = res.tile([128, 3 * NL], F32, tag="dinvb")     # [.,:NL]=dinv
            stage = res.tile([128, NL], FP8, tag="stage")         # g2 fp8 stage
            consts = res.tile([128, 144], F32, tag="consts")
            cbf = res.tile([128, 256], BF16, tag="cbf")
            smalls = res.tile([128, 8], F32, tag="smalls")
            ident = res.tile([128, 128], F32, tag="ident")
            make_identity(nc, ident[:])

            nc.sync.dma_start(consts[:], d_consts[:])
            nc.sync.dma_start(cbf[:], d_cbf[:])
            nc.sync.dma_start(dinvb[:], d_dinvb[:])

            dv = dinvb[:, 0:NL]            # dinv
            dv8 = dinvb[:, NL:2 * NL]      # dinv / G_SCALE
            dv64 = dinvb[:, 2 * NL:3 * NL]  # dinv / (XT_SCALE*G_SCALE)

            for _rep in range(reps):
             with (tc.For_i(0, loops, 1) if loops else nullcontext()):
              # A tiles into resident buffer
              for t in range(8):
                eng = nc.sync if t % 2 == 0 else nc.scalar
                eng.dma_start(a_res[:, t * 8192:(t + 1) * 8192], d_a[t])

              w1_ap = cbf[:, CB_W1:CB_W1 + 128]
              w2_ap = cbf[:, CB_W2:CB_W2 + 128]

              # ------- phase 1: G1 = xs_bf @ (8*W1), agg1 -------
              out1 = psum.tile([128, NL], F32, tag="out1")

              a3 = a_res[:].rearrange("p (c d) -> p c d", d=1024)
              DR = mybir.MatmulPerfMode.DoubleRow
              for j in range(2):            # 2 xt tiles of 32 chunks each
                  xt = stream.tile([128, 4096], FP8, tag="xt")
                  nc.sync.dma_start(xt[:], d_xt[j])
                  for qq in range(32):
                      q = j * 32 + qq
                      pxw = psum2.tile([128, 128], F32, tag="mm_small")
                      nc.tensor.matmul(
                          pxw[:],
                          xt[:, qq * 128:(qq + 1) * 128],
                          w1_ap,
                          start=True, stop=True,
                      )
                      ghi_s = g_res[:, q * 128:(q + 1) * 128]
                      nc.vector.tensor_copy(ghi_s, pxw[:])
                      # agg1 for chunk pair (q-1, q): fp8 DoubleRow
                      if q % 2 == 1:
                          gp = g_res[:, (q - 1) * 128:(q + 1) * 128].rearrange(
                              "p (two f) -> p two f", two=2)
                          for b in range(2):
                              nc.tensor.matmul(
                                  out1[:, b * 512:(b + 1) * 512],
                                  gp,
                                  a3[:, q - 1:q + 1, b * 512:(b + 1) * 512],
                                  start=(q == 1),
                                  stop=(q == SC - 1),
                                  perf_mode=DR,
                              )

              # h1s = dinv * relu(out1*dinv/64 + b1)
              nc.vector.tensor_mul(h1_sb[:], out1[:], dv64)
              nc.scalar.activation(h1_sb[:], h1_sb[:],
                                   mybir.ActivationFunctionType.Relu,
                                   bias=consts[:, C_B1:C_B1 + 1], scale=1.0)
              nc.vector.tensor_mul(h1_sb[:], h1_sb[:], dv)

              # ------- phase 1.5: G2 = h1s^T @ (8*W2) -> fp8 -> AllGather ---
              h1_bf = e_bf  # scratch reuse: [128, NL] bf16
              nc.vector.tensor_copy(h1_bf[:], h1_sb[:])
              for m in range(8):
                  pg2 = psum2.tile([128, 128], F32, tag="mm_small")
                  nc.tensor.matmul(
                      pg2[:],
                      h1_bf[:, m * 128:(m + 1) * 128],
                      w2_ap,
                      start=True, stop=True,
                  )
                  nc.vector.tensor_copy(stage[:, m * 128:(m + 1) * 128], pg2[:])
              ag_in_r = ag_in.ap().rearrange("(m p) f -> p m f", p=128)
              nc.sync.dma_start(ag_in_r,
                                stage[:, 0:NL].rearrange("p (m f) -> p m f", f=128))
              if not loops:
                  nc.gpsimd.collective_compute(
                      "AllGather", mybir.AluOpType.bypass,
                      replica_groups=[list(range(N_CORES))],
                      ins=[ag_in.ap().opt()],
                      outs=[ag_out.ap().opt()],
                  )
              # DMA gathered G2 (fp8) straight into g_res, node-major chunks
              for j in range(4):
                  src = ag_out[j * 2048:(j + 1) * 2048, :].rearrange(
                      "(c p) f -> p c f", p=128)
                  dstv = g_res[:, j * 2048:(j + 1) * 2048].rearrange(
                      "p (c f) -> p c f", f=128)
                  nc.sync.dma_start(dstv, src)

              # ------- phase 2: agg2 (fp8 DoubleRow, shared LDW) + e + fc1 ---
              pa2 = psum.tile([128, NL], F32, tag="out1")
              for q in range(0, SC, 2):
                  gp = g_res[:, q * 128:(q + 2) * 128].rearrange(
                      "p (two f) -> p two f", two=2)
                  for b in range(2):
                      nc.tensor.matmul(
                          pa2[:, b * 512:(b + 1) * 512],
                          gp,
                          a3[:, q:q + 2, b * 512:(b + 1) * 512],
                          start=(q == 0), stop=(q == SC - 2),
                          perf_mode=DR,
                      )
              # e_bf = bf16(pa2 * dinv/8 + b2)
              nc.vector.tensor_mul(e_bf[:], pa2[:], dv8)
              nc.vector.tensor_scalar_add(e_bf[:], e_bf[:],
                                          consts[:, C_B2:C_B2 + 1])

              # fc1 (e_bf bf16 x wf fp8, psum x WF_SCALE)
              pfc = psum.tile([4, 512], F32, tag="pfc")
              n_fc1_mm = 0
              total_fc1_mm = NL // 4
              for wt in range(WF_TILES):
                  wtile = stream.tile([128, WF_TILE], FP8, tag="wf")
                  eng = nc.sync if wt % 2 == 0 else nc.gpsimd
                  eng.dma_start(wtile[:], d_wf[wt])
                  for g in range(WF_TILE // 512):      # 32 quads per tile
                      nb = wt * (WF_TILE // D) + g * 4
                      nc.tensor.matmul(
                          pfc[:],
                          e_bf[:, nb:nb + 4],
                          wtile[:, g * 512:(g + 1) * 512],
                          start=(n_fc1_mm == 0),
                          stop=(n_fc1_mm == total_fc1_mm - 1),
                      )
                      n_fc1_mm += 1

              # ------- tail: diag extract via PE transpose, AllReduce, fc2/fc3
              s4 = res.tile([4, 512], F32, tag="s4")
              nc.vector.tensor_copy(s4[:], pfc[:])
              ptr = psum2.tile([128, 16], F32, tag="pa2")
              for m in range(4):
                  nc.tensor.transpose(ptr[:, m * 4:(m + 1) * 4],
                                      s4[:, m * 128:(m + 1) * 128],
                                      ident[0:4, 0:4])
              part0 = res.tile([128, 1], F32, tag="part0")
              nc.vector.tensor_copy(part0[:], ptr[:, 0:1])
              nc.vector.tensor_add(part0[:], part0[:], ptr[:, 5:6])
              nc.vector.tensor_add(part0[:], part0[:], ptr[:, 10:11])
              nc.vector.tensor_add(part0[:], part0[:], ptr[:, 15:16])
              nc.sync.dma_start(ar_in.ap().rearrange("o p -> p o"), part0[:])
              if not loops:
                  nc.gpsimd.collective_compute(
                      "AllReduce", mybir.AluOpType.add,
                      replica_groups=[list(range(N_CORES))],
                      ins=[ar_in.ap().opt()],
                      outs=[ar_out.ap().opt()],
                  )
              hcol = smalls[:, 0:1]
              nc.sync.dma_start(hcol, ar_out.ap().rearrange("o p -> p o"))
              nc.scalar.activation(hcol, hcol,
                                   mybir.ActivationFunctionType.Relu,
                                   bias=consts[:, C_FC1B:C_FC1B + 1],
                                   scale=1.0 / WF_SCALE)
              ph2 = psum2.tile([128, 1], F32, tag="mm_small")
              nc.tensor.matmul(ph2[:], consts[:, C_FC2W:C_FC2W + 128], hcol,
                               start=True, stop=True)
              h2col = smalls[:, 1:2]
              nc.scalar.activation(h2col, ph2[:],
                                   mybir.ActivationFunctionType.Relu,
                                   bias=consts[:, C_FC2B:C_FC2B + 1], scale=1.0)
              pq = psum2.tile([128, 1], F32, tag="mm_small")
              nc.tensor.matmul(pq[0:1, 0:1], consts[:, C_FC3W:C_FC3W + 1], h2col,
                               start=True, stop=True)
              qt = res.tile([1, 1], F32, tag="qt")
              nc.vector.tensor_add(qt[:], pq[0:1, 0:1],
                                   consts[0:1, C_FC3B:C_FC3B + 1])
              nc.sync.dma_start(d_q[:], qt[:])

    nc.compile()
    _NC_CACHE[key] = nc
    return nc


def preprocess(x, edge_index, W1, b1, W2, b2, fc1_w, fc1_b, fc2_w, fc2_b,
               fc3_w, fc3_b):
    """Host-side index preprocessing + data layout. Returns per-core in_maps."""
    BF = ml_dtypes.bfloat16
    F8 = ml_dtypes.float8_e4m3
    x = np.asarray(x, np.float32)
    ei = np.asarray(edge_index)
    src = ei[0].astype(np.int64)
    dst = ei[1].astype(np.int64)

    deg = (np.bincount(dst, minlength=N) + 1).astype(np.float32)
    dinv = (1.0 / np.sqrt(deg)).astype(np.float32)

    # dense adjacency counts + self loops (exact small ints)
    A = np.zeros((N, N), np.float32)
    np.add.at(A, (src, dst), 1.0)
    A[np.arange(N), np.arange(N)] += 1.0

    xs = x * (dinv[:, None] * XT_SCALE)          # fold deg^-1/2 into x rows
    xsT = np.ascontiguousarray(xs.T)             # [128, 8192]
    xt_tiles = np.ascontiguousarray(
        xsT.reshape(128, 2, 4096).transpose(1, 0, 2)).astype(F8)

    wf3 = np.asarray(fc1_w, np.float32).reshape(N, D, D) * WF_SCALE

    consts = np.zeros((128, 144), np.float32)
    consts[:, 0:128] = fc2_w
    consts[:, 128] = fc3_w[:, 0]
    consts[:, 129] = b1
    consts[:, 130] = b2
    consts[:, 131] = fc1_b
    consts[:, 132] = fc2_b
    consts[:, 133] = fc3_b[0]
    cbf = np.zeros((128, 256), np.float32)
    cbf[:, 0:128] = W1 * G_SCALE
    cbf[:, 128:256] = W2 * G_SCALE
    cbf = cbf.astype(BF)

    in_maps = []
    for c in range(N_CORES):
        sl = slice(c * NL, (c + 1) * NL)
        a_c = A[:, sl]                                        # [8192, 1024]
        a_tiles = np.ascontiguousarray(
            a_c.reshape(8, 8, 128, NL).transpose(0, 2, 1, 3)
               .reshape(8, 128, 8192)).astype(F8)
        wf_c = np.ascontiguousarray(
            wf3[sl].transpose(1, 0, 2).reshape(128, WF_TILES, WF_TILE)
                   .transpose(1, 0, 2)).astype(F8)            # [8,128,16384]
        dinvb_c = np.zeros((128, 3 * NL), np.float32)
        dinvb_c[:, 0:NL] = dinv[sl][None, :]
        dinvb_c[:, NL:2 * NL] = dinv[sl][None, :] / G_SCALE
        dinvb_c[:, 2 * NL:3 * NL] = dinv[sl][None, :] / (G_SCALE * XT_SCALE)
        in_maps.append({
            "xt": xt_tiles,
            "ablk": a_tiles,
            "wf": wf_c,
            "dinvb": dinvb_c,
            "consts": consts,
            "cbf": cbf,
        })
    return in_maps


def kernel(**inputs):
    in_maps = preprocess(**inputs)
    nc = build_nc()
    br = run_bass_kernel_spmd(nc, in_maps, core_ids=list(range(N_CORES)))
    q = np.asarray(br.results[0]["out_q"], np.float32).reshape(1)
    return q


def kernel_traced(inputs, trace=False, reps=1):
    """For test.py: returns (q, BassKernelResults)."""
    in_maps = preprocess(**inputs)
    nc = build_nc(reps=reps)
    br = run_bass_kernel_spmd(nc, in_maps, core_ids=list(range(N_CORES)),
                              trace=trace)
    q = np.asarray(br.results[0]["out_q"], np.float32).reshape(1)
    return q, br


# revision 15
# speedup vs baseline: 1.0969x; 1.0969x over previous
"""GCN(2-layer) + DQN-head kernel for 8 TRN2 NeuronCores.

v2: fc1 weights streamed as fp8e4m3 scaled by 2^12 (halves the dominant HBM
stream; the inverse scale folds into the tail activation). W1/W2 scaled by 8
so G1/G2 fp8 values sit in e4m3's normal range (host sim: rel err 1.3e-3 vs
fp32 reference). fc1 matmul is mixed bf16 (e) x fp8 (wf).

Per core c of 8 (owns dst nodes [c*1024, (c+1)*1024)):
  host:   fold deg^-1/2 into x rows; dense (Adj+I) count block
          A[src=8192, dst=1024] fp8; fc1_w slice * 4096 -> fp8 tiles.
  device: G1[src,feat] = xs_bf @ (8*W1)           (fp8 result, x8 scale)
          out1[feat,dst] = sum_src G1[src,:]*A[src,dst]   (A resident)
          h1s = dinvb * relu(out1*dinvb/8 + b1)
          G2 = h1s^T @ (8*W2) -> fp8 -> AllGather -> g_res
          agg2 per 256-dst group; e_bf = bf16(psum*dinvb/8 + b2)
          fc1: [K=128,M=4]x[128,512] e_bf x wf_fp8 matmuls, psum x4096
          diag extract -> AllReduce(128) -> relu(x/4096+b)/fc2/fc3 -> q
"""
import sys
import numpy as np

sys.path.insert(0, "/opt/trn_rl_repo")

import ml_dtypes
import concourse.bass as bass
import concourse.tile as tile
import concourse.mybir as mybir
from concourse import bacc
from concourse.bass_utils import run_bass_kernel_spmd
from concourse.masks import make_identity

F32 = mybir.dt.float32
BF16 = mybir.dt.bfloat16
FP8 = mybir.dt.float8e4

N_CORES = 8
N = 8192
D = 128
NL = N // N_CORES          # 1024 local dst nodes
SC = N // 128              # 64 src chunks
WF_TILE = 16384            # wf tile free dim (128 nodes/tile, fp8 -> 2MB)
WF_TILES = NL * D // WF_TILE   # 8 tiles per core
GROUPS = 4                 # dst groups for agg2/fc1 interleave
GDST = NL // GROUPS        # 256 dst per group

WF_SCALE = 4096.0          # fc1_w stored as fp8(w * 4096)
G_SCALE = 8.0              # W1/W2 stored as bf16(8*W); G2 fp8 holds 8x values
XT_SCALE = 8.0             # x*dinv stored as fp8(8*xs); G1 fp8 holds 64x

_NC_CACHE = {}


def build_nc(reps=1, loops=None, probe=None):
    key = (reps, loops, probe)
    if key in _NC_CACHE:
        return _NC_CACHE[key]
    nc = bacc.Bacc("TRN2", target_bir_lowering=False, debug=False,
                   num_devices=N_CORES)

    # ---------------- DRAM I/O ----------------
    d_xt = nc.dram_tensor("xt", [2, 128, 4096], FP8, kind="ExternalInput")
    d_a = nc.dram_tensor("ablk", [8, 128, 8192], FP8, kind="ExternalInput")
    d_wf = nc.dram_tensor("wf", [WF_TILES, 128, WF_TILE], FP8,
                          kind="ExternalInput")
    d_dinvb = nc.dram_tensor("dinvb", [128, 3 * NL], F32, kind="ExternalInput")
    d_consts = nc.dram_tensor("consts", [128, 144], F32, kind="ExternalInput")
    d_cbf = nc.dram_tensor("cbf", [128, 256], BF16, kind="ExternalInput")
    d_q = nc.dram_tensor("out_q", [1, 1], F32, kind="ExternalOutput")

    # collective bounce buffers
    ag_in = nc.dram_tensor("ag_in", [NL, D], FP8)
    ag_out = nc.dram_tensor("ag_out", [N, D], FP8, addr_space="Shared")
    ar_in = nc.dram_tensor("ar_in", [1, 128], F32)
    ar_out = nc.dram_tensor("ar_out", [1, 128], F32, addr_space="Shared")

    # consts (f32) columns
    C_FC2W = 0      # [128, 128]
    C_FC3W = 128    # [128, 1]
    C_B1 = 129
    C_B2 = 130
    C_FC1B = 131
    C_FC2B = 132
    C_FC3B = 133    # replicated; use [0:1]
    # cbf (bf16) columns
    CB_W1 = 0       # [128, 128] (8*W1)
    CB_W2 = 128     # [128, 128] (8*W2)

    with tile.TileContext(nc) as tc:
        from contextlib import ExitStack, nullcontext
        with ExitStack() as ctx:
            P = bass.MemorySpace.PSUM
            res = ctx.enter_context(tc.tile_pool(name="res", bufs=1))
            stream = ctx.enter_context(tc.tile_pool(name="stream", bufs=3))
            streamw = ctx.enter_context(tc.tile_pool(name="streamw", bufs=4))
            psum = ctx.enter_context(tc.tile_pool(name="psum", bufs=1, space=P))
            psum2 = ctx.enter_context(tc.tile_pool(name="psum2", bufs=2, space=P))

            # ------- resident tensors -------
            a_res = res.tile([128, SC * NL], FP8, tag="a_res")    # 64KB/part
            g_res = res.tile([128, N], FP8, tag="g_res")          # 8KB/part
            e_bf = res.tile([128, NL], BF16, tag="e_bf")
            h1_sb = res.tile([128, NL], F32, tag="h1")
            dinvb = res.tile([128, 3 * NL], F32, tag="dinvb")     # [.,:NL]=dinv
            stage = res.tile([128, NL], FP8, tag="stage")         # g2 fp8 stage
            consts = res.tile([128, 144], F32, tag="consts")
            cbf = res.tile([128, 256], BF16, tag="cbf")
            smalls = res.tile([128, 8], F32, tag="smalls")
            ident = res.tile([128, 128], F32, tag="ident")
            make_identity(nc, ident[:])

            nc.sync.dma_start(consts[:], d_consts[:])
            nc.sync.dma_start(cbf[:], d_cbf[:])
            nc.sync.dma_start(dinvb[:], d_dinvb[:])

            dv = dinvb[:, 0:NL]            # dinv
            dv8 = dinvb[:, NL:2 * NL]      # dinv / G_SCALE
            dv64 = dinvb[:, 2 * NL:3 * NL]  # dinv / (XT_SCALE*G_SCALE)

            for _rep in range(reps):
             with (tc.For_i(0, loops, 1) if loops else nullcontext()):
              xts = []
              for j in range(2):
                  xt = stream.tile([128, 4096], FP8, tag="xt")
                  if probe != "compute":
                      nc.sync.dma_start(xt[:], d_xt[j])
                  xts.append(xt)
              # A tiles into resident buffer
              if probe != "compute":
                for t in range(8):
                  eng = nc.sync if t % 2 == 0 else nc.scalar
                  eng.dma_start(a_res[:, t * 8192:(t + 1) * 8192], d_a[t])

              w1_ap = cbf[:, CB_W1:CB_W1 + 128]
              w2_ap = cbf[:, CB_W2:CB_W2 + 128]

              # ------- phase 1: G1 = xs_bf @ (8*W1), agg1 -------
              out1 = psum.tile([128, NL], F32, tag="out1")

              a3 = a_res[:].rearrange("p (c d) -> p c d", d=1024)
              DR = mybir.MatmulPerfMode.DoubleRow
              for j in range(2):            # 2 xt tiles of 32 chunks each
                  xt = xts[j]
                  for qq in range(32):
                      if probe == "dma":
                          continue
                      q = j * 32 + qq
                      pxw = psum2.tile([128, 128], F32, tag="mm_small")
                      nc.tensor.matmul(
                          pxw[:],
                          xt[:, qq * 128:(qq + 1) * 128],
                          w1_ap,
                          start=True, stop=True,
                      )
                      ghi_s = g_res[:, q * 128:(q + 1) * 128]
                      nc.vector.tensor_copy(ghi_s, pxw[:])
                      # agg1 for chunk pair (q-1, q): fp8 DoubleRow
                      if q % 2 == 1:
                          gp = g_res[:, (q - 1) * 128:(q + 1) * 128].rearrange(
                              "p (two f) -> p two f", two=2)
                          for b in range(2):
                              nc.tensor.matmul(
                                  out1[:, b * 512:(b + 1) * 512],
                                  gp,
                                  a3[:, q - 1:q + 1, b * 512:(b + 1) * 512],
                                  start=(q == 1),
                                  stop=(q == SC - 1),
                                  perf_mode=DR,
                              )

              # h1s = dinv * relu(out1*dinv/64 + b1)
              if probe != "dma":
                nc.vector.tensor_mul(h1_sb[:], out1[:], dv64)
                nc.scalar.activation(h1_sb[:], h1_sb[:],
                                     mybir.ActivationFunctionType.Relu,
                                     bias=consts[:, C_B1:C_B1 + 1], scale=1.0)
                nc.vector.tensor_mul(h1_sb[:], h1_sb[:], dv)

              # ------- phase 1.5: G2 = h1s^T @ (8*W2) -> fp8 -> AllGather ---
              h1_bf = e_bf  # scratch reuse: [128, NL] bf16
              if probe != "dma":
                nc.vector.tensor_copy(h1_bf[:], h1_sb[:])
                for m in range(8):
                  pg2 = psum2.tile([128, 128], F32, tag="mm_small")
                  nc.tensor.matmul(
                      pg2[:],
                      h1_bf[:, m * 128:(m + 1) * 128],
                      w2_ap,
                      start=True, stop=True,
                  )
                  nc.vector.tensor_copy(stage[:, m * 128:(m + 1) * 128], pg2[:])
              if probe is None:
                  ag_in_r = ag_in.ap().rearrange("(m p) f -> p m f", p=128)
                  nc.sync.dma_start(ag_in_r,
                                    stage[:, 0:NL].rearrange("p (m f) -> p m f", f=128))
              if not loops:
                  nc.gpsimd.collective_compute(
                      "AllGather", mybir.AluOpType.bypass,
                      replica_groups=[list(range(N_CORES))],
                      ins=[ag_in.ap().opt()],
                      outs=[ag_out.ap().opt()],
                  )
              # DMA gathered G2 (fp8) straight into g_res, node-major chunks
              if probe != "compute":
                for j in range(4):
                  src = ag_out[j * 2048:(j + 1) * 2048, :].rearrange(
                      "(c p) f -> p c f", p=128)
                  dstv = g_res[:, j * 2048:(j + 1) * 2048].rearrange(
                      "p (c f) -> p c f", f=128)
                  nc.sync.dma_start(dstv, src)

              # ------- phase 2: agg2 (fp8 DoubleRow, shared LDW) + e + fc1 ---
              pa2 = psum.tile([128, NL], F32, tag="out1")
              for q in range(0, SC, 2):
                  if probe == "dma":
                      continue
                  gp = g_res[:, q * 128:(q + 2) * 128].rearrange(
                      "p (two f) -> p two f", two=2)
                  for b in range(2):
                      nc.tensor.matmul(
                          pa2[:, b * 512:(b + 1) * 512],
                          gp,
                          a3[:, q:q + 2, b * 512:(b + 1) * 512],
                          start=(q == 0), stop=(q == SC - 2),
                          perf_mode=DR,
                      )
              # e_bf = bf16(pa2 * dinv/8 + b2)
              if probe != "dma":
                nc.vector.tensor_mul(e_bf[:], pa2[:], dv8)
                nc.vector.tensor_scalar_add(e_bf[:], e_bf[:],
                                            consts[:, C_B2:C_B2 + 1])

              # fc1 (e_bf bf16 x wf fp8, psum x WF_SCALE)
              pfc = psum.tile([4, 512], F32, tag="pfc")
              n_fc1_mm = 0
              total_fc1_mm = NL // 4
              for wt in range(WF_TILES):
                  wtile = streamw.tile([128, WF_TILE], FP8, tag="wf")
                  if probe != "compute":
                      eng = nc.sync if wt % 2 == 0 else nc.gpsimd
                      eng.dma_start(wtile[:], d_wf[wt])
                  for g in range(WF_TILE // 512):      # 32 quads per tile
                      if probe == "dma":
                          continue
                      nb = wt * (WF_TILE // D) + g * 4
                      nc.tensor.matmul(
                          pfc[:],
                          e_bf[:, nb:nb + 4],
                          wtile[:, g * 512:(g + 1) * 512],
                          start=(n_fc1_mm == 0),
                          stop=(n_fc1_mm == total_fc1_mm - 1),
                      )
                      n_fc1_mm += 1

              # ------- tail: diag extract via PE transpose, AllReduce, fc2/fc3
              s4 = res.tile([4, 512], F32, tag="s4")
              if probe != "dma":
                nc.vector.tensor_copy(s4[:], pfc[:])
              ptr = psum2.tile([128, 16], F32, tag="pa2")
              part0 = res.tile([128, 1], F32, tag="part0")
              if probe != "dma":
                for m in range(4):
                  nc.tensor.transpose(ptr[:, m * 4:(m + 1) * 4],
                                      s4[:, m * 128:(m + 1) * 128],
                                      ident[0:4, 0:4])
                nc.vector.tensor_copy(part0[:], ptr[:, 0:1])
                nc.vector.tensor_add(part0[:], part0[:], ptr[:, 5:6])
                nc.vector.tensor_add(part0[:], part0[:], ptr[:, 10:11])
                nc.vector.tensor_add(part0[:], part0[:], ptr[:, 15:16])
              if probe != "dma":
                  nc.sync.dma_start(ar_in.ap().rearrange("o p -> p o"), part0[:])
              if not loops:
                  nc.gpsimd.collective_compute(
                      "AllReduce", mybir.AluOpType.add,
                      replica_groups=[list(range(N_CORES))],
                      ins=[ar_in.ap().opt()],
                      outs=[ar_out.ap().opt()],
                  )
              hcol = smalls[:, 0:1]
              nc.sync.dma_start(hcol, ar_out.ap().rearrange("o p -> p o"))
              nc.scalar.activation(hcol, hcol,
                                   mybir.ActivationFunctionType.Relu,
                                   bias=consts[:, C_FC1B:C_FC1B + 1],
                                   scale=1.0 / WF_SCALE)
              ph2 = psum2.tile([128, 1], F32, tag="mm_small")
              nc.tensor.matmul(ph2[:], consts[:, C_FC2W:C_FC2W + 128], hcol,
                               start=True, stop=True)
              h2col = smalls[:, 1:2]
              nc.scalar.activation(h2col, ph2[:],
                                   mybir.ActivationFunctionType.Relu,
                                   bias=consts[:, C_FC2B:C_FC2B + 1], scale=1.0)
              pq = psum2.tile([128, 1], F32, tag="mm_small")
              nc.tensor.matmul(pq[0:1, 0:1], consts[:, C_FC3W:C_FC3W + 1], h2col,
                               start=True, stop=True)
              qt = res.tile([1, 1], F32, tag="qt")
              nc.vector.tensor_add(qt[:], pq[0:1, 0:1],
                                   consts[0:1, C_FC3B:C_FC3B + 1])
              nc.sync.dma_start(d_q[:], qt[:])

    nc.compile()
    _NC_CACHE[key] = nc
    return nc


def preprocess(x, edge_index, W1, b1, W2, b2, fc1_w, fc1_b, fc2_w, fc2_b,
               fc3_w, fc3_b):
    """Host-side index preprocessing + data layout. Returns per-core in_maps."""
    BF = ml_dtypes.bfloat16
    F8 = ml_dtypes.float8_e4m3
    x = np.asarray(x, np.float32)
    ei = np.asarray(edge_index)
    src = ei[0].astype(np.int64)
    dst = ei[1].astype(np.int64)

    deg = (np.bincount(dst, minlength=N) + 1).astype(np.float32)
    dinv = (1.0 / np.sqrt(deg)).astype(np.float32)

    # dense adjacency counts + self loops (exact small ints)
    A = np.zeros((N, N), np.float32)
    np.add.at(A, (src, dst), 1.0)
    A[np.arange(N), np.arange(N)] += 1.0

    xs = x * (dinv[:, None] * XT_SCALE)          # fold deg^-1/2 into x rows
    xsT = np.ascontiguousarray(xs.T)             # [128, 8192]
    xt_tiles = np.ascontiguousarray(
        xsT.reshape(128, 2, 4096).transpose(1, 0, 2)).astype(F8)

    wf3 = np.asarray(fc1_w, np.float32).reshape(N, D, D) * WF_SCALE

    consts = np.zeros((128, 144), np.float32)
    consts[:, 0:128] = fc2_w
    consts[:, 128] = fc3_w[:, 0]
    consts[:, 129] = b1
    consts[:, 130] = b2
    consts[:, 131] = fc1_b
    consts[:, 132] = fc2_b
    consts[:, 133] = fc3_b[0]
    cbf = np.zeros((128, 256), np.float32)
    cbf[:, 0:128] = W1 * G_SCALE
    cbf[:, 128:256] = W2 * G_SCALE
    cbf = cbf.astype(BF)

    in_maps = []
    for c in range(N_CORES):
        sl = slice(c * NL, (c + 1) * NL)
        a_c = A[:, sl]                                        # [8192, 1024]
        a_tiles = np.ascontiguousarray(
            a_c.reshape(8, 8, 128, NL).transpose(0, 2, 1, 3)
               .reshape(8, 128, 8192)).astype(F8)
        wf_c = np.ascontiguousarray(
            wf3[sl].transpose(1, 0, 2).reshape(128, WF_TILES, WF_TILE)
                   .transpose(1, 0, 2)).astype(F8)            # [8,128,16384]
        dinvb_c = np.zeros((128, 3 * NL), np.float32)
        dinvb_c[:, 0:NL] = dinv[sl][None, :]
        dinvb_c[:, NL:2 * NL] = dinv[sl][None, :] / G_SCALE
        dinvb_c[:, 2 * NL:3 * NL] = dinv[sl][None, :] / (G_SCALE * XT_SCALE)
        in_maps.append({
            "xt": xt_tiles,
            "ablk": a_tiles,
            "wf": wf_c,
            "dinvb": dinvb_c,
            "consts": consts,
            "cbf": cbf,
        })
    return in_maps


def kernel(**inputs):
    in_maps = preprocess(**inputs)
    nc = build_nc()
    br = run_bass_kernel_spmd(nc, in_maps, core_ids=list(range(N_CORES)))
    q = np.asarray(br.results[0]["out_q"], np.float32).reshape(1)
    return q


def kernel_traced(inputs, trace=False, reps=1):
    """For test.py: returns (q, BassKernelResults)."""
    in_maps = preprocess(**inputs)
    nc = build_nc(reps=reps)
    br = run_bass_kernel_spmd(nc, in_maps, core_ids=list(range(N_CORES)),
                              trace=trace)
    q = np.asarray(br.results[0]["out_q"], np.float32).reshape(1)
    return q, br


# revision 17
# speedup vs baseline: 1.1016x; 1.0043x over previous
"""GCN(2-layer) + DQN-head kernel for 8 TRN2 NeuronCores.

v2: fc1 weights streamed as fp8e4m3 scaled by 2^12 (halves the dominant HBM
stream; the inverse scale folds into the tail activation). W1/W2 scaled by 8
so G1/G2 fp8 values sit in e4m3's normal range (host sim: rel err 1.3e-3 vs
fp32 reference). fc1 matmul is mixed bf16 (e) x fp8 (wf).

Per core c of 8 (owns dst nodes [c*1024, (c+1)*1024)):
  host:   fold deg^-1/2 into x rows; dense (Adj+I) count block
          A[src=8192, dst=1024] fp8; fc1_w slice * 4096 -> fp8 tiles.
  device: G1[src,feat] = xs_bf @ (8*W1)           (fp8 result, x8 scale)
          out1[feat,dst] = sum_src G1[src,:]*A[src,dst]   (A resident)
          h1s = dinvb * relu(out1*dinvb/8 + b1)
          G2 = h1s^T @ (8*W2) -> fp8 -> AllGather -> g_res
          agg2 per 256-dst group; e_bf = bf16(psum*dinvb/8 + b2)
          fc1: [K=128,M=4]x[128,512] e_bf x wf_fp8 matmuls, psum x4096
          diag extract -> AllReduce(128) -> relu(x/4096+b)/fc2/fc3 -> q
"""
import sys
import numpy as np

sys.path.insert(0, "/opt/trn_rl_repo")

import ml_dtypes
import concourse.bass as bass
import concourse.tile as tile
import concourse.mybir as mybir
from concourse import bacc
from concourse.bass_utils import run_bass_kernel_spmd
from concourse.masks import make_identity

F32 = mybir.dt.float32
BF16 = mybir.dt.bfloat16
FP8 = mybir.dt.float8e4

N_CORES = 8
N = 8192
D = 128
NL = N // N_CORES          # 1024 local dst nodes
SC = N // 128              # 64 src chunks
WF_TILE = 16384            # wf tile free dim (128 nodes/tile, fp8 -> 2MB)
WF_TILES = NL * D // WF_TILE   # 8 tiles per core
GROUPS = 4                 # dst groups for agg2/fc1 interleave
GDST = NL // GROUPS        # 256 dst per group

WF_SCALE = 4096.0          # fc1_w stored as fp8(w * 4096)
G_SCALE = 8.0              # W1/W2 stored as bf16(8*W); G2 fp8 holds 8x values
XT_SCALE = 8.0             # x*dinv stored as fp8(8*xs); G1 fp8 holds 64x

_NC_CACHE = {}


def build_nc(reps=1, loops=None, probe=None):
    key = (reps, loops, probe)
    if key in _NC_CACHE:
        return _NC_CACHE[key]
    nc = bacc.Bacc("TRN2", target_bir_lowering=False, debug=False,
                   num_devices=N_CORES)

    # ---------------- DRAM I/O ----------------
    d_xt = nc.dram_tensor("xt", [2, 128, 4096], FP8, kind="ExternalInput")
    d_a = nc.dram_tensor("ablk", [8, 128, 8192], FP8, kind="ExternalInput")
    d_wf = nc.dram_tensor("wf", [WF_TILES, 128, WF_TILE], FP8,
                          kind="ExternalInput")
    d_dinvb = nc.dram_tensor("dinvb", [128, 3 * NL], F32, kind="ExternalInput")
    d_consts = nc.dram_tensor("consts", [128, 144], F32, kind="ExternalInput")
    d_cbf = nc.dram_tensor("cbf", [128, 256], BF16, kind="ExternalInput")
    d_q = nc.dram_tensor("out_q", [1, 1], F32, kind="ExternalOutput")

    # collective bounce buffers
    ag_in = nc.dram_tensor("ag_in", [NL, D], FP8)
    ag_out = nc.dram_tensor("ag_out", [N, D], FP8, addr_space="Shared")
    ar_in = nc.dram_tensor("ar_in", [1, 128], F32)
    ar_out = nc.dram_tensor("ar_out", [1, 128], F32, addr_space="Shared")

    # consts (f32) columns
    C_FC2W = 0      # [128, 128]
    C_FC3W = 128    # [128, 1]
    C_B1 = 129
    C_B2 = 130
    C_FC1B = 131
    C_FC2B = 132
    C_FC3B = 133    # replicated; use [0:1]
    # cbf (bf16) columns
    CB_W1 = 0       # [128, 128] (8*W1)
    CB_W2 = 128     # [128, 128] (8*W2)

    with tile.TileContext(nc) as tc:
        from contextlib import ExitStack, nullcontext
        with ExitStack() as ctx:
            P = bass.MemorySpace.PSUM
            res = ctx.enter_context(tc.tile_pool(name="res", bufs=1))
            stream = ctx.enter_context(tc.tile_pool(name="stream", bufs=3))
            streamw = ctx.enter_context(tc.tile_pool(name="streamw", bufs=5))
            psum = ctx.enter_context(tc.tile_pool(name="psum", bufs=1, space=P))
            psum2 = ctx.enter_context(tc.tile_pool(name="psum2", bufs=2, space=P))

            # ------- resident tensors -------
            a_res = res.tile([128, SC * NL], FP8, tag="a_res")    # 64KB/part
            g_res = res.tile([128, N], FP8, tag="g_res")          # 8KB/part
            e_bf = res.tile([128, NL], BF16, tag="e_bf")
            h1_sb = res.tile([128, NL], F32, tag="h1")
            dinvb = res.tile([128, 3 * NL], F32, tag="dinvb")     # [.,:NL]=dinv
            stage = res.tile([128, NL], FP8, tag="stage")         # g2 fp8 stage
            consts = res.tile([128, 144], F32, tag="consts")
            cbf = res.tile([128, 256], BF16, tag="cbf")
            smalls = res.tile([128, 8], F32, tag="smalls")
            ident = res.tile([128, 128], F32, tag="ident")
            make_identity(nc, ident[:])

            nc.sync.dma_start(consts[:], d_consts[:])
            nc.sync.dma_start(cbf[:], d_cbf[:])
            nc.sync.dma_start(dinvb[:], d_dinvb[:])

            dv = dinvb[:, 0:NL]            # dinv
            dv8 = dinvb[:, NL:2 * NL]      # dinv / G_SCALE
            dv64 = dinvb[:, 2 * NL:3 * NL]  # dinv / (XT_SCALE*G_SCALE)

            for _rep in range(reps):
             with (tc.For_i(0, loops, 1) if loops else nullcontext()):
              xts = []
              for j in range(2):
                  xt = stream.tile([128, 4096], FP8, tag="xt")
                  if probe != "compute":
                      nc.sync.dma_start(xt[:], d_xt[j])
                  xts.append(xt)
              # A tiles into resident buffer
              if probe != "compute":
                for t in range(8):
                  eng = nc.sync if t % 2 == 0 else nc.scalar
                  eng.dma_start(a_res[:, t * 8192:(t + 1) * 8192], d_a[t])

              w1_ap = cbf[:, CB_W1:CB_W1 + 128]
              w2_ap = cbf[:, CB_W2:CB_W2 + 128]

              # ------- phase 1: G1 = xs_bf @ (8*W1), agg1 -------
              out1 = psum.tile([128, NL], F32, tag="out1")

              a3 = a_res[:].rearrange("p (c d) -> p c d", d=1024)
              DR = mybir.MatmulPerfMode.DoubleRow
              for j in range(2):            # 2 xt tiles of 32 chunks each
                  xt = xts[j]
                  for qq in range(32):
                      if probe == "dma":
                          continue
                      q = j * 32 + qq
                      pxw = psum2.tile([128, 128], F32, tag="mm_small")
                      nc.tensor.matmul(
                          pxw[:],
                          xt[:, qq * 128:(qq + 1) * 128],
                          w1_ap,
                          start=True, stop=True,
                      )
                      ghi_s = g_res[:, q * 128:(q + 1) * 128]
                      nc.vector.tensor_copy(ghi_s, pxw[:])
                      # agg1 for chunk pair (q-1, q): fp8 DoubleRow
                      if q % 2 == 1:
                          gp = g_res[:, (q - 1) * 128:(q + 1) * 128].rearrange(
                              "p (two f) -> p two f", two=2)
                          for b in range(2):
                              nc.tensor.matmul(
                                  out1[:, b * 512:(b + 1) * 512],
                                  gp,
                                  a3[:, q - 1:q + 1, b * 512:(b + 1) * 512],
                                  start=(q == 1),
                                  stop=(q == SC - 1),
                                  perf_mode=DR,
                              )

              # h1s = dinv * relu(out1*dinv/64 + b1)
              if probe != "dma":
                nc.vector.tensor_mul(h1_sb[:], out1[:], dv64)
                nc.scalar.activation(h1_sb[:], h1_sb[:],
                                     mybir.ActivationFunctionType.Relu,
                                     bias=consts[:, C_B1:C_B1 + 1], scale=1.0)
                nc.vector.tensor_mul(h1_sb[:], h1_sb[:], dv)

              # ------- phase 1.5: G2 = h1s^T @ (8*W2) -> fp8 -> AllGather ---
              h1_bf = e_bf  # scratch reuse: [128, NL] bf16
              if probe != "dma":
                nc.vector.tensor_copy(h1_bf[:], h1_sb[:])
                for m in range(8):
                  pg2 = psum2.tile([128, 128], F32, tag="mm_small")
                  nc.tensor.matmul(
                      pg2[:],
                      h1_bf[:, m * 128:(m + 1) * 128],
                      w2_ap,
                      start=True, stop=True,
                  )
                  nc.vector.tensor_copy(stage[:, m * 128:(m + 1) * 128], pg2[:])
              if probe is None:
                  ag_in_r = ag_in.ap().rearrange("(m p) f -> p m f", p=128)
                  nc.sync.dma_start(ag_in_r,
                                    stage[:, 0:NL].rearrange("p (m f) -> p m f", f=128))
              if not loops:
                  nc.gpsimd.collective_compute(
                      "AllGather", mybir.AluOpType.bypass,
                      replica_groups=[list(range(N_CORES))],
                      ins=[ag_in.ap().opt()],
                      outs=[ag_out.ap().opt()],
                  )
              # DMA gathered G2 (fp8) straight into g_res, node-major chunks
              if probe != "compute":
                for j in range(4):
                  src = ag_out[j * 2048:(j + 1) * 2048, :].rearrange(
                      "(c p) f -> p c f", p=128)
                  dstv = g_res[:, j * 2048:(j + 1) * 2048].rearrange(
                      "p (c f) -> p c f", f=128)
                  nc.scalar.dma_start(dstv, src)

              # ------- phase 2: agg2 (fp8 DoubleRow, shared LDW) + e + fc1 ---
              pa2 = psum.tile([128, NL], F32, tag="out1")
              for q in range(0, SC, 2):
                  if probe == "dma":
                      continue
                  gp = g_res[:, q * 128:(q + 2) * 128].rearrange(
                      "p (two f) -> p two f", two=2)
                  for b in range(2):
                      nc.tensor.matmul(
                          pa2[:, b * 512:(b + 1) * 512],
                          gp,
                          a3[:, q:q + 2, b * 512:(b + 1) * 512],
                          start=(q == 0), stop=(q == SC - 2),
                          perf_mode=DR,
                      )
              # e_bf = bf16(pa2 * dinv/8 + b2)
              if probe != "dma":
                nc.vector.tensor_mul(e_bf[:], pa2[:], dv8)
                nc.vector.tensor_scalar_add(e_bf[:], e_bf[:],
                                            consts[:, C_B2:C_B2 + 1])

              # fc1 (e_bf bf16 x wf fp8, psum x WF_SCALE)
              pfc = psum.tile([4, 512], F32, tag="pfc")
              n_fc1_mm = 0
              total_fc1_mm = NL // 4
              for wt in range(WF_TILES):
                  wtile = streamw.tile([128, WF_TILE], FP8, tag="wf")
                  if probe != "compute":
                      eng = nc.sync if wt % 2 == 0 else nc.gpsimd
                      eng.dma_start(wtile[:], d_wf[wt])
                  for g in range(WF_TILE // 512):      # 32 quads per tile
                      if probe == "dma":
                          continue
                      nb = wt * (WF_TILE // D) + g * 4
                      nc.tensor.matmul(
                          pfc[:],
                          e_bf[:, nb:nb + 4],
                          wtile[:, g * 512:(g + 1) * 512],
                          start=(n_fc1_mm == 0),
                          stop=(n_fc1_mm == total_fc1_mm - 1),
                      )
                      n_fc1_mm += 1

              # ------- tail: diag extract via PE transpose, AllReduce, fc2/fc3
              s4 = res.tile([4, 512], F32, tag="s4")
              if probe != "dma":
                nc.vector.tensor_copy(s4[:], pfc[:])
              ptr = psum2.tile([128, 16], F32, tag="pa2")
              part0 = res.tile([128, 1], F32, tag="part0")
              if probe != "dma":
                for m in range(4):
                  nc.tensor.transpose(ptr[:, m * 4:(m + 1) * 4],
                                      s4[:, m * 128:(m + 1) * 128],
                                      ident[0:4, 0:4])
                nc.vector.tensor_copy(part0[:], ptr[:, 0:1])
                nc.vector.tensor_add(part0[:], part0[:], ptr[:, 5:6])
                nc.vector.tensor_add(part0[:], part0[:], ptr[:, 10:11])
                nc.vector.tensor_add(part0[:], part0[:], ptr[:, 15:16])
              if probe != "dma":
                  nc.sync.dma_start(ar_in.ap().rearrange("o p -> p o"), part0[:])
              if not loops:
                  nc.gpsimd.collective_compute(
                      "AllReduce", mybir.AluOpType.add,
                      replica_groups=[list(range(N_CORES))],
                      ins=[ar_in.ap().opt()],
                      outs=[ar_out.ap().opt()],
                  )
              hcol = smalls[:, 0:1]
              nc.sync.dma_start(hcol, ar_out.ap().rearrange("o p -> p o"))
              nc.scalar.activation(hcol, hcol,
                                   mybir.ActivationFunctionType.Relu,
                                   bias=consts[:, C_FC1B:C_FC1B + 1],
                                   scale=1.0 / WF_SCALE)
              ph2 = psum2.tile([128, 1], F32, tag="mm_small")
              nc.tensor.matmul(ph2[:], consts[:, C_FC2W:C_FC2W + 128], hcol,
                               start=True, stop=True)
              h2col = smalls[:, 1:2]
              nc.scalar.activation(h2col, ph2[:],
                                   mybir.ActivationFunctionType.Relu,
                                   bias=consts[:, C_FC2B:C_FC2B + 1], scale=1.0)
              pq = psum2.tile([128, 1], F32, tag="mm_small")
              nc.tensor.matmul(pq[0:1, 0:1], consts[:, C_FC3W:C_FC3W + 1], h2col,
                               start=True, stop=True)
              qt = res.tile([1, 1], F32, tag="qt")
              nc.vector.tensor_add(qt[:], pq[0:1, 0:1],
                                   consts[0:1, C_FC3B:C_FC3B + 1])
              nc.sync.dma_start(d_q[:], qt[:])

    nc.compile()
    _NC_CACHE[key] = nc
    return nc


def preprocess(x, edge_index, W1, b1, W2, b2, fc1_w, fc1_b, fc2_w, fc2_b,
               fc3_w, fc3_b):
    """Host-side index preprocessing + data layout. Returns per-core in_maps."""
    BF = ml_dtypes.bfloat16
    F8 = ml_dtypes.float8_e4m3
    x = np.asarray(x, np.float32)
    ei = np.asarray(edge_index)
    src = ei[0].astype(np.int64)
    dst = ei[1].astype(np.int64)

    deg = (np.bincount(dst, minlength=N) + 1).astype(np.float32)
    dinv = (1.0 / np.sqrt(deg)).astype(np.float32)

    # dense adjacency counts + self loops (exact small ints)
    A = np.zeros((N, N), np.float32)
    np.add.at(A, (src, dst), 1.0)
    A[np.arange(N), np.arange(N)] += 1.0

    xs = x * (dinv[:, None] * XT_SCALE)          # fold deg^-1/2 into x rows
    xsT = np.ascontiguousarray(xs.T)             # [128, 8192]
    xt_tiles = np.ascontiguousarray(
        xsT.reshape(128, 2, 4096).transpose(1, 0, 2)).astype(F8)

    wf3 = np.asarray(fc1_w, np.float32).reshape(N, D, D) * WF_SCALE

    consts = np.zeros((128, 144), np.float32)
    consts[:, 0:128] = fc2_w
    consts[:, 128] = fc3_w[:, 0]
    consts[:, 129] = b1
    consts[:, 130] = b2
    consts[:, 131] = fc1_b
    consts[:, 132] = fc2_b
    consts[:, 133] = fc3_b[0]
    cbf = np.zeros((128, 256), np.float32)
    cbf[:, 0:128] = W1 * G_SCALE
    cbf[:, 128:256] = W2 * G_SCALE
    cbf = cbf.astype(BF)

    in_maps = []
    for c in range(N_CORES):
        sl = slice(c * NL, (c + 1) * NL)
        a_c = A[:, sl]                                        # [8192, 1024]
        a_tiles = np.ascontiguousarray(
            a_c.reshape(8, 8, 128, NL).transpose(0, 2, 1, 3)
               .reshape(8, 128, 8192)).astype(F8)
        wf_c = np.ascontiguousarray(
            wf3[sl].transpose(1, 0, 2).reshape(128, WF_TILES, WF_TILE)
                   .transpose(1, 0, 2)).astype(F8)            # [8,128,16384]
        dinvb_c = np.zeros((128, 3 * NL), np.float32)
        dinvb_c[:, 0:NL] = dinv[sl][None, :]
        dinvb_c[:, NL:2 * NL] = dinv[sl][None, :] / G_SCALE
        dinvb_c[:, 2 * NL:3 * NL] = dinv[sl][None, :] / (G_SCALE * XT_SCALE)
        in_maps.append({
            "xt": xt_tiles,
            "ablk": a_tiles,
            "wf": wf_c,
            "dinvb": dinvb_c,
            "consts": consts,
            "cbf": cbf,
        })
    return in_maps


def kernel(**inputs):
    in_maps = preprocess(**inputs)
    nc = build_nc()
    br = run_bass_kernel_spmd(nc, in_maps, core_ids=list(range(N_CORES)))
    q = np.asarray(br.results[0]["out_q"], np.float32).reshape(1)
    return q


def kernel_traced(inputs, trace=False, reps=1):
    """For test.py: returns (q, BassKernelResults)."""
    in_maps = preprocess(**inputs)
    nc = build_nc(reps=reps)
    br = run_bass_kernel_spmd(nc, in_maps, core_ids=list(range(N_CORES)),
                              trace=trace)
    q = np.asarray(br.results[0]["out_q"], np.float32).reshape(1)
    return q, br


# revision 18
# speedup vs baseline: 1.1154x; 1.0125x over previous
"""GCN(2-layer) + DQN-head kernel for 8 TRN2 NeuronCores.

v2: fc1 weights streamed as fp8e4m3 scaled by 2^12 (halves the dominant HBM
stream; the inverse scale folds into the tail activation). W1/W2 scaled by 8
so G1/G2 fp8 values sit in e4m3's normal range (host sim: rel err 1.3e-3 vs
fp32 reference). fc1 matmul is mixed bf16 (e) x fp8 (wf).

Per core c of 8 (owns dst nodes [c*1024, (c+1)*1024)):
  host:   fold deg^-1/2 into x rows; dense (Adj+I) count block
          A[src=8192, dst=1024] fp8; fc1_w slice * 4096 -> fp8 tiles.
  device: G1[src,feat] = xs_bf @ (8*W1)           (fp8 result, x8 scale)
          out1[feat,dst] = sum_src G1[src,:]*A[src,dst]   (A resident)
          h1s = dinvb * relu(out1*dinvb/8 + b1)
          G2 = h1s^T @ (8*W2) -> fp8 -> AllGather -> g_res
          agg2 per 256-dst group; e_bf = bf16(psum*dinvb/8 + b2)
          fc1: [K=128,M=4]x[128,512] e_bf x wf_fp8 matmuls, psum x4096
          diag extract -> AllReduce(128) -> relu(x/4096+b)/fc2/fc3 -> q
"""
import sys
import numpy as np

sys.path.insert(0, "/opt/trn_rl_repo")

import ml_dtypes
import concourse.bass as bass
import concourse.tile as tile
import concourse.mybir as mybir
from concourse import bacc
from concourse.bass_utils import run_bass_kernel_spmd
from concourse.masks import make_identity

F32 = mybir.dt.float32
BF16 = mybir.dt.bfloat16
FP8 = mybir.dt.float8e4

N_CORES = 8
N = 8192
D = 128
NL = N // N_CORES          # 1024 local dst nodes
SC = N // 128              # 64 src chunks
WF_TILE = 16384            # wf tile free dim (128 nodes/tile, fp8 -> 2MB)
WF_TILES = NL * D // WF_TILE   # 8 tiles per core
GROUPS = 4                 # dst groups for agg2/fc1 interleave
GDST = NL // GROUPS        # 256 dst per group

WF_SCALE = 4096.0          # fc1_w stored as fp8(w * 4096)
G_SCALE = 8.0              # W1/W2 stored as bf16(8*W); G2 fp8 holds 8x values
XT_SCALE = 8.0             # x*dinv stored as fp8(8*xs); G1 fp8 holds 64x

_NC_CACHE = {}


def build_nc(reps=1, loops=None, probe=None):
    key = (reps, loops, probe)
    if key in _NC_CACHE:
        return _NC_CACHE[key]
    nc = bacc.Bacc("TRN2", target_bir_lowering=False, debug=False,
                   num_devices=N_CORES)

    # ---------------- DRAM I/O ----------------
    d_xt = nc.dram_tensor("xt", [2, 128, 4096], FP8, kind="ExternalInput")
    d_a = nc.dram_tensor("ablk", [4, 128, 16384], FP8, kind="ExternalInput")
    d_wf = nc.dram_tensor("wf", [WF_TILES, 128, WF_TILE], FP8,
                          kind="ExternalInput")
    d_dinvb = nc.dram_tensor("dinvb", [128, 3 * NL], F32, kind="ExternalInput")
    d_consts = nc.dram_tensor("consts", [128, 144], F32, kind="ExternalInput")
    d_cbf = nc.dram_tensor("cbf", [128, 256], BF16, kind="ExternalInput")
    d_q = nc.dram_tensor("out_q", [1, 1], F32, kind="ExternalOutput")

    # collective bounce buffers
    ag_in = nc.dram_tensor("ag_in", [NL, D], FP8)
    ag_out = nc.dram_tensor("ag_out", [N, D], FP8, addr_space="Shared")
    ar_in = nc.dram_tensor("ar_in", [1, 128], F32)
    ar_out = nc.dram_tensor("ar_out", [1, 128], F32, addr_space="Shared")

    # consts (f32) columns
    C_FC2W = 0      # [128, 128]
    C_FC3W = 128    # [128, 1]
    C_B1 = 129
    C_B2 = 130
    C_FC1B = 131
    C_FC2B = 132
    C_FC3B = 133    # replicated; use [0:1]
    # cbf (bf16) columns
    CB_W1 = 0       # [128, 128] (8*W1)
    CB_W2 = 128     # [128, 128] (8*W2)

    with tile.TileContext(nc) as tc:
        from contextlib import ExitStack, nullcontext
        with ExitStack() as ctx:
            P = bass.MemorySpace.PSUM
            res = ctx.enter_context(tc.tile_pool(name="res", bufs=1))
            stream = ctx.enter_context(tc.tile_pool(name="stream", bufs=3))
            streamw = ctx.enter_context(tc.tile_pool(name="streamw", bufs=5))
            psum = ctx.enter_context(tc.tile_pool(name="psum", bufs=1, space=P))
            psum2 = ctx.enter_context(tc.tile_pool(name="psum2", bufs=2, space=P))

            # ------- resident tensors -------
            a_res = res.tile([128, SC * NL], FP8, tag="a_res")    # 64KB/part
            g_res = res.tile([128, N], FP8, tag="g_res")          # 8KB/part
            e_bf = res.tile([128, NL], BF16, tag="e_bf")
            h1_sb = res.tile([128, NL], F32, tag="h1")
            dinvb = res.tile([128, 3 * NL], F32, tag="dinvb")     # [.,:NL]=dinv
            stage = res.tile([128, NL], FP8, tag="stage")         # g2 fp8 stage
            consts = res.tile([128, 144], F32, tag="consts")
            cbf = res.tile([128, 256], BF16, tag="cbf")
            smalls = res.tile([128, 8], F32, tag="smalls")
            ident = res.tile([128, 128], F32, tag="ident")
            make_identity(nc, ident[:])

            nc.sync.dma_start(consts[:], d_consts[:])
            nc.sync.dma_start(cbf[:], d_cbf[:])
            nc.sync.dma_start(dinvb[:], d_dinvb[:])

            dv = dinvb[:, 0:NL]            # dinv
            dv8 = dinvb[:, NL:2 * NL]      # dinv / G_SCALE
            dv64 = dinvb[:, 2 * NL:3 * NL]  # dinv / (XT_SCALE*G_SCALE)

            for _rep in range(reps):
             with (tc.For_i(0, loops, 1) if loops else nullcontext()):
              xts = []
              for j in range(2):
                  xt = stream.tile([128, 4096], FP8, tag="xt")
                  if probe != "compute":
                      nc.sync.dma_start(xt[:], d_xt[j])
                  xts.append(xt)
              # A tiles into resident buffer
              if probe != "compute":
                for t in range(4):
                  eng = nc.sync if t % 2 == 0 else nc.scalar
                  eng.dma_start(a_res[:, t * 16384:(t + 1) * 16384], d_a[t])

              w1_ap = cbf[:, CB_W1:CB_W1 + 128]
              w2_ap = cbf[:, CB_W2:CB_W2 + 128]

              # ------- phase 1: G1 = xs_bf @ (8*W1), agg1 -------
              out1 = psum.tile([128, NL], F32, tag="out1")

              a3 = a_res[:].rearrange("p (c d) -> p c d", d=1024)
              DR = mybir.MatmulPerfMode.DoubleRow
              for j in range(2):            # 2 xt tiles of 32 chunks each
                  xt = xts[j]
                  for qq in range(32):
                      if probe == "dma":
                          continue
                      q = j * 32 + qq
                      pxw = psum2.tile([128, 128], F32, tag="mm_small")
                      nc.tensor.matmul(
                          pxw[:],
                          xt[:, qq * 128:(qq + 1) * 128],
                          w1_ap,
                          start=True, stop=True,
                      )
                      ghi_s = g_res[:, q * 128:(q + 1) * 128]
                      nc.vector.tensor_copy(ghi_s, pxw[:])
                      # agg1 for chunk pair (q-1, q): fp8 DoubleRow
                      if q % 2 == 1:
                          gp = g_res[:, (q - 1) * 128:(q + 1) * 128].rearrange(
                              "p (two f) -> p two f", two=2)
                          for b in range(2):
                              nc.tensor.matmul(
                                  out1[:, b * 512:(b + 1) * 512],
                                  gp,
                                  a3[:, q - 1:q + 1, b * 512:(b + 1) * 512],
                                  start=(q == 1),
                                  stop=(q == SC - 1),
                                  perf_mode=DR,
                              )

              # h1s = dinv * relu(out1*dinv/64 + b1)
              if probe != "dma":
                nc.vector.tensor_mul(h1_sb[:], out1[:], dv64)
                nc.scalar.activation(h1_sb[:], h1_sb[:],
                                     mybir.ActivationFunctionType.Relu,
                                     bias=consts[:, C_B1:C_B1 + 1], scale=1.0)
                nc.vector.tensor_mul(h1_sb[:], h1_sb[:], dv)

              # ------- phase 1.5: G2 = h1s^T @ (8*W2) -> fp8 -> AllGather ---
              h1_bf = e_bf  # scratch reuse: [128, NL] bf16
              if probe != "dma":
                nc.vector.tensor_copy(h1_bf[:], h1_sb[:])
                for m in range(8):
                  pg2 = psum2.tile([128, 128], F32, tag="mm_small")
                  nc.tensor.matmul(
                      pg2[:],
                      h1_bf[:, m * 128:(m + 1) * 128],
                      w2_ap,
                      start=True, stop=True,
                  )
                  nc.vector.tensor_copy(stage[:, m * 128:(m + 1) * 128], pg2[:])
              if probe is None:
                  ag_in_r = ag_in.ap().rearrange("(m p) f -> p m f", p=128)
                  nc.scalar.dma_start(ag_in_r,
                                    stage[:, 0:NL].rearrange("p (m f) -> p m f", f=128))
              if not loops:
                  nc.gpsimd.collective_compute(
                      "AllGather", mybir.AluOpType.bypass,
                      replica_groups=[list(range(N_CORES))],
                      ins=[ag_in.ap().opt()],
                      outs=[ag_out.ap().opt()],
                  )
              # DMA gathered G2 (fp8) straight into g_res, node-major chunks
              if probe != "compute":
                for j in range(4):
                  src = ag_out[j * 2048:(j + 1) * 2048, :].rearrange(
                      "(c p) f -> p c f", p=128)
                  dstv = g_res[:, j * 2048:(j + 1) * 2048].rearrange(
                      "p (c f) -> p c f", f=128)
                  nc.scalar.dma_start(dstv, src)

              # ------- phase 2: agg2 (fp8 DoubleRow, shared LDW) + e + fc1 ---
              pa2 = psum.tile([128, NL], F32, tag="out1")
              for q in range(0, SC, 2):
                  if probe == "dma":
                      continue
                  gp = g_res[:, q * 128:(q + 2) * 128].rearrange(
                      "p (two f) -> p two f", two=2)
                  for b in range(2):
                      nc.tensor.matmul(
                          pa2[:, b * 512:(b + 1) * 512],
                          gp,
                          a3[:, q:q + 2, b * 512:(b + 1) * 512],
                          start=(q == 0), stop=(q == SC - 2),
                          perf_mode=DR,
                      )
              # e_bf = bf16(pa2 * dinv/8 + b2)
              if probe != "dma":
                nc.vector.tensor_mul(e_bf[:], pa2[:], dv8)
                nc.vector.tensor_scalar_add(e_bf[:], e_bf[:],
                                            consts[:, C_B2:C_B2 + 1])

              # fc1 (e_bf bf16 x wf fp8, psum x WF_SCALE)
              pfc = psum.tile([4, 512], F32, tag="pfc")
              n_fc1_mm = 0
              total_fc1_mm = NL // 4
              for wt in range(WF_TILES):
                  wtile = streamw.tile([128, WF_TILE], FP8, tag="wf")
                  if probe != "compute":
                      eng = nc.sync if wt % 2 == 0 else nc.gpsimd
                      eng.dma_start(wtile[:], d_wf[wt])
                  for g in range(WF_TILE // 512):      # 32 quads per tile
                      if probe == "dma":
                          continue
                      nb = wt * (WF_TILE // D) + g * 4
                      nc.tensor.matmul(
                          pfc[:],
                          e_bf[:, nb:nb + 4],
                          wtile[:, g * 512:(g + 1) * 512],
                          start=(n_fc1_mm == 0),
                          stop=(n_fc1_mm == total_fc1_mm - 1),
                      )
                      n_fc1_mm += 1

              # ------- tail: diag extract via PE transpose, AllReduce, fc2/fc3
              s4 = res.tile([4, 512], F32, tag="s4")
              if probe != "dma":
                nc.vector.tensor_copy(s4[:], pfc[:])
              ptr = psum2.tile([128, 16], F32, tag="pa2")
              part0 = res.tile([128, 1], F32, tag="part0")
              if probe != "dma":
                for m in range(4):
                  nc.tensor.transpose(ptr[:, m * 4:(m + 1) * 4],
                                      s4[:, m * 128:(m + 1) * 128],
                                      ident[0:4, 0:4])
                nc.vector.tensor_copy(part0[:], ptr[:, 0:1])
                nc.vector.tensor_add(part0[:], part0[:], ptr[:, 5:6])
                nc.vector.tensor_add(part0[:], part0[:], ptr[:, 10:11])
                nc.vector.tensor_add(part0[:], part0[:], ptr[:, 15:16])
              if probe != "dma":
                  nc.sync.dma_start(ar_in.ap().rearrange("o p -> p o"), part0[:])
              if not loops:
                  nc.gpsimd.collective_compute(
                      "AllReduce", mybir.AluOpType.add,
                      replica_groups=[list(range(N_CORES))],
                      ins=[ar_in.ap().opt()],
                      outs=[ar_out.ap().opt()],
                  )
              hcol = smalls[:, 0:1]
              nc.sync.dma_start(hcol, ar_out.ap().rearrange("o p -> p o"))
              nc.scalar.activation(hcol, hcol,
                                   mybir.ActivationFunctionType.Relu,
                                   bias=consts[:, C_FC1B:C_FC1B + 1],
                                   scale=1.0 / WF_SCALE)
              ph2 = psum2.tile([128, 1], F32, tag="mm_small")
              nc.tensor.matmul(ph2[:], consts[:, C_FC2W:C_FC2W + 128], hcol,
                               start=True, stop=True)
              h2col = smalls[:, 1:2]
              nc.scalar.activation(h2col, ph2[:],
                                   mybir.ActivationFunctionType.Relu,
                                   bias=consts[:, C_FC2B:C_FC2B + 1], scale=1.0)
              pq = psum2.tile([128, 1], F32, tag="mm_small")
              nc.tensor.matmul(pq[0:1, 0:1], consts[:, C_FC3W:C_FC3W + 1], h2col,
                               start=True, stop=True)
              qt = res.tile([1, 1], F32, tag="qt")
              nc.vector.tensor_add(qt[:], pq[0:1, 0:1],
                                   consts[0:1, C_FC3B:C_FC3B + 1])
              nc.sync.dma_start(d_q[:], qt[:])

    nc.compile()
    _NC_CACHE[key] = nc
    return nc


def preprocess(x, edge_index, W1, b1, W2, b2, fc1_w, fc1_b, fc2_w, fc2_b,
               fc3_w, fc3_b):
    """Host-side index preprocessing + data layout. Returns per-core in_maps."""
    BF = ml_dtypes.bfloat16
    F8 = ml_dtypes.float8_e4m3
    x = np.asarray(x, np.float32)
    ei = np.asarray(edge_index)
    src = ei[0].astype(np.int64)
    dst = ei[1].astype(np.int64)

    deg = (np.bincount(dst, minlength=N) + 1).astype(np.float32)
    dinv = (1.0 / np.sqrt(deg)).astype(np.float32)

    # dense adjacency counts + self loops (exact small ints)
    A = np.zeros((N, N), np.float32)
    np.add.at(A, (src, dst), 1.0)
    A[np.arange(N), np.arange(N)] += 1.0

    xs = x * (dinv[:, None] * XT_SCALE)          # fold deg^-1/2 into x rows
    xsT = np.ascontiguousarray(xs.T)             # [128, 8192]
    xt_tiles = np.ascontiguousarray(
        xsT.reshape(128, 2, 4096).transpose(1, 0, 2)).astype(F8)

    wf3 = np.asarray(fc1_w, np.float32).reshape(N, D, D) * WF_SCALE

    consts = np.zeros((128, 144), np.float32)
    consts[:, 0:128] = fc2_w
    consts[:, 128] = fc3_w[:, 0]
    consts[:, 129] = b1
    consts[:, 130] = b2
    consts[:, 131] = fc1_b
    consts[:, 132] = fc2_b
    consts[:, 133] = fc3_b[0]
    cbf = np.zeros((128, 256), np.float32)
    cbf[:, 0:128] = W1 * G_SCALE
    cbf[:, 128:256] = W2 * G_SCALE
    cbf = cbf.astype(BF)

    in_maps = []
    for c in range(N_CORES):
        sl = slice(c * NL, (c + 1) * NL)
        a_c = A[:, sl]                                        # [8192, 1024]
        a_tiles = np.ascontiguousarray(
            a_c.reshape(4, 16, 128, NL).transpose(0, 2, 1, 3)
               .reshape(4, 128, 16384)).astype(F8)
        wf_c = np.ascontiguousarray(
            wf3[sl].transpose(1, 0, 2).reshape(128, WF_TILES, WF_TILE)
                   .transpose(1, 0, 2)).astype(F8)            # [8,128,16384]
        dinvb_c = np.zeros((128, 3 * NL), np.float32)
        dinvb_c[:, 0:NL] = dinv[sl][None, :]
        dinvb_c[:, NL:2 * NL] = dinv[sl][None, :] / G_SCALE
        dinvb_c[:, 2 * NL:3 * NL] = dinv[sl][None, :] / (G_SCALE * XT_SCALE)
        in_maps.append({
            "xt": xt_tiles,
            "ablk": a_tiles,
            "wf": wf_c,
            "dinvb": dinvb_c,
            "consts": consts,
            "cbf": cbf,
        })
    return in_maps


def kernel(**inputs):
    in_maps = preprocess(**inputs)
    nc = build_nc()
    br = run_bass_kernel_spmd(nc, in_maps, core_ids=list(range(N_CORES)))
    q = np.asarray(br.results[0]["out_q"], np.float32).reshape(1)
    return q


def kernel_traced(inputs, trace=False, reps=1):
    """For test.py: returns (q, BassKernelResults)."""
    in_maps = preprocess(**inputs)
    nc = build_nc(reps=reps)
    br = run_bass_kernel_spmd(nc, in_maps, core_ids=list(range(N_CORES)),
                              trace=trace)
    q = np.asarray(br.results[0]["out_q"], np.float32).reshape(1)
    return q, br
